# revision 26
# baseline (speedup 1.0000x reference)
"""AffinityPropagate prediction kernel for Trainium2 (8 NeuronCores).

Data-parallel over batch B=8: each core owns one image [480, 640].

Column-major layout per core: host transposes images to [W=640, H=480];
128 partitions x 5 columns each (exact fit). Each column slot is padded
to 482 (zero row at both ends). State kept in fp32 (single in-place
buffer, own columns only). Each iteration builds scaled fp16 copies fb
(and fbs = fb shifted one element) -- double-buffered -- with left/right
halo column slots refreshed by SBUF->SBUF DMA.

Per iteration (matches reference math):
  f_new = w_center*f + sum_{8 taps} w_t * shift_t(f)
Center product + final add run in fp32 (dominates accuracy); the 8
other products and their pairwise-tree sum run in fp16 at DVE 2x mode,
on values scaled by 2^-iter to stay inside fp16 range (|w_t| <= 1
guarantees no overflow; the unscale folds into the final fused op).
Ops are emitted in 2 row-groups so ACT copies + halo DMAs overlap DVE.
"""

import numpy as np
from contextlib import ExitStack

import concourse.bacc as bacc
import concourse.mybir as mybir
import concourse.tile as tile
from concourse import bass_utils
from concourse.bass_interp import get_hw_module

B, CH, H, W = 8, 8, 480, 640
P = 128            # partitions (each holds COLS_P image columns)
COLS_P = W // P    # 5
CSLOT = H + 2      # col slot: [0, rows at 1..480, 0]
NSLOT = COLS_P + 2  # fb/fbs: + left/right halo column slots
PX = COLS_P * H    # 2400 compact px per partition
RG = H // 2        # row-group size (240)

F32 = mybir.dt.float32
FP16 = mybir.dt.float16
AF = mybir.ActivationFunctionType
OP = mybir.AluOpType

# 8 non-center taps in reference slab order: (dr, dc); w8b slab i = TAPS8[i]
TAPS8 = [(t // 3 - 1, t % 3 - 1) for t in range(9) if t != 4]


def _build(times: int):
    nc = bacc.Bacc("TRN2", debug=False, dynamic_dma_scratch_size=2048)
    # host passes column-major (transposed) data
    aff_d = nc.dram_tensor("affinity", [CH, W * H], F32, kind="ExternalInput")
    feat_d = nc.dram_tensor("feature", [W * H], F32, kind="ExternalInput")
    out_d = nc.dram_tensor("out", [W * H], F32, kind="ExternalOutput")

    with tile.TileContext(nc) as tc, ExitStack() as ctx:
        pool = ctx.enter_context(tc.tile_pool(name="main", bufs=1))

        w8b = pool.tile([P, 8, PX], FP16)          # fp16 tap weights
        w4f = pool.tile([P, PX], F32)              # fp32 center weight
        f32s = pool.tile([P, COLS_P * CSLOT], F32)  # fp32 state (in-place)
        fbp = [pool.tile([P, NSLOT * CSLOT], FP16, name=f"fb{i}")
               for i in range(2)]
        fbsp = [pool.tile([P, NSLOT * CSLOT], FP16, name=f"fbs{i}")
                for i in range(2)]
        # fp16 scratch for products/tree, per row-group
        pr = [[pool.tile([P, COLS_P * RG], FP16, name=f"pr{g}_{i}")
               for i in range(4)] for g in range(2)]
        sums = pool.tile([P, PX], F32)
        suma = pool.tile([P, PX], F32)
        rec = pool.tile([P, PX], F32)
        rec16 = pool.tile([P, PX], FP16)
        # scr shares slots with the per-iteration c32 ring
        scr = pool.tile([P, PX], F32, tag="c32", bufs=2)

        def f3(t, w):
            return t[:, :].rearrange("p (s x) -> p s x", x=w)

        fs3 = f3(f32s, CSLOT)                      # [P, 5, 482]
        fbv = [f3(t, CSLOT) for t in fbp]          # [P, 7, 482]
        fbsv = [f3(t, CSLOT) for t in fbsp]

        # ---- zero-init padded buffers ----
        nc.vector.memset(f32s[:, :], 0.0)
        for t in fbp + fbsp:
            nc.vector.memset(t[:, :], 0.0)

        # ---- load feature (transposed [W,H] -> [P,5,480]) ----
        feat_v = feat_d[:].rearrange("(p c r) -> p c r", c=COLS_P, r=H)
        nc.sync.dma_start(fs3[:, :, 1:1 + H], feat_v)

        # ---- load affinity (channel at a time) + weight prep ----
        aff_v = aff_d[:, :].rearrange("c (p x) -> c p x", x=PX)
        for c in range(CH):
            st = pool.tile([P, PX], F32, name=f"stage{c}", tag="stg", bufs=2)
            nc.sync.dma_start(st[:, :], aff_v[c])
            # fp16 copy of channel into weight slab (normalized later)
            nc.scalar.activation(w8b[:, c, :], st[:, :], AF.Copy)
            if c == 0:
                nc.scalar.activation(sums[:, :], st[:, :], AF.Abs)
                nc.vector.tensor_copy(suma[:, :], st[:, :])
            else:
                absdst = rec if c % 2 else w4f
                nc.scalar.activation(absdst[:, :], st[:, :], AF.Abs)
                nc.vector.tensor_add(sums[:, :], sums[:, :], absdst[:, :])
                nc.vector.tensor_add(suma[:, :], suma[:, :], st[:, :])
        nc.vector.reciprocal_approx_accurate(rec[:, :], sums[:, :], scr[:, :])
        # fp16 copy of rec so slab normalization runs in DVE 2x mode
        nc.scalar.activation(rec16[:, :], rec[:, :], AF.Copy)
        for i in range(8):
            nc.vector.tensor_mul(w8b[:, i, :], w8b[:, i, :], rec16[:, :])
        # center = 1 - suma * rec  (fp32)
        nc.vector.scalar_tensor_tensor(w4f[:, :], suma[:, :], -1.0, rec[:, :],
                                       OP.mult, OP.mult)
        nc.vector.tensor_scalar_add(w4f[:, :], w4f[:, :], 1.0)

        def fb_copies(dfb, scale, g):
            # fp16 scaled copy of f32 state rows of group g into dfb
            r0 = g * RG
            nc.scalar.activation(dfb[:, 1:1 + COLS_P, 1 + r0:1 + r0 + RG],
                                 fs3[:, :, 1 + r0:1 + r0 + RG], AF.Copy,
                                 scale=scale)

        def fbs_copy(dfbs, scale, g):
            # fbs[x] = fb[x+1]: shifted scaled copy, split at the group
            # seam (dst elems [0,239) from G0 rows, [239,481) from G1)
            if g == 0:
                nc.scalar.activation(dfbs[:, 1:1 + COLS_P, 0:RG - 1],
                                     fs3[:, :, 1:RG], AF.Copy, scale=scale)
            else:
                nc.scalar.activation(dfbs[:, 1:1 + COLS_P, RG - 1:H + 1],
                                     fs3[:, :, RG:CSLOT], AF.Copy,
                                     scale=scale)

        def fb_halo_dmas(dfb, g):
            r0 = g * RG
            sl = slice(1 + r0, 1 + r0 + RG)
            nc.sync.dma_start(dfb[1:P, 0, sl], dfb[0:P - 1, COLS_P, sl])
            nc.sync.dma_start(dfb[0:P - 1, NSLOT - 1, sl], dfb[1:P, 1, sl])

        def fbs_halo_dmas(dfbs, g):
            sl = slice(0, RG - 1) if g == 0 else slice(RG - 1, H + 1)
            nc.sync.dma_start(dfbs[1:P, 0, sl], dfbs[0:P - 1, COLS_P, sl])
            nc.sync.dma_start(dfbs[0:P - 1, NSLOT - 1, sl], dfbs[1:P, 1, sl])

        for g in (0, 1):
            fb_copies(fbv[0], 1.0, g)
            fb_halo_dmas(fbv[0], g)
            fbs_copy(fbsv[0], 1.0, g)
            fbs_halo_dmas(fbsv[0], g)

        wv = w8b[:, :, :].rearrange("p s (c r) -> p s c r", r=H)
        w4v = w4f[:, :].rearrange("p (c r) -> p c r", r=H)
        out_v = out_d[:].rearrange("(p c r) -> p c r", c=COLS_P, r=H)

        # ---- iterations ----
        for it in range(times):
            cfb, cfbs = fbv[it % 2], fbsv[it % 2]
            nfb, nfbs = fbv[(it + 1) % 2], fbsv[(it + 1) % 2]
            for g in (0, 1):
                r0 = g * RG
                a, b, c_, d = pr[g]

                def mul8(dst, k):
                    dr, dc = TAPS8[k]
                    wvg = wv[:, k, :, r0:r0 + RG]
                    if dr == 0:
                        # misaligned in fb; aligned via fbs (fbs[x]=fb[x+1])
                        src = cfbs[:, 1 + dc:1 + dc + COLS_P, r0:r0 + RG]
                    else:
                        src = cfb[:, 1 + dc:1 + dc + COLS_P,
                                  1 + r0 + dr:1 + r0 + dr + RG]
                    nc.vector.tensor_mul(
                        dst[:, :].rearrange("p (c r) -> p c r", r=RG),
                        src, wvg)

                def add2(dst, x, y):
                    nc.vector.tensor_add(dst[:, :], x[:, :], y[:, :])

                # fbs-dependent taps (k=3,4: dr==0) go last so the shifted
                # copy + its halo DMAs can complete under earlier DVE work
                mul8(a, 0); mul8(b, 1); add2(a, a, b)
                mul8(b, 2); mul8(c_, 5); add2(b, b, c_)
                add2(a, a, b)
                mul8(b, 6); mul8(c_, 7); add2(b, b, c_)
                mul8(c_, 3); mul8(d, 4); add2(c_, c_, d)
                add2(b, b, c_)
                add2(a, a, b)
                # center product fp32: c32 = w4f * f32
                c32 = pool.tile([P, COLS_P * RG], F32, name=f"c32_{it}_{g}",
                                tag="c32", bufs=2)
                c32v = c32[:, :].rearrange("p (c r) -> p c r", r=RG)
                nc.vector.tensor_mul(c32v, fs3[:, :, 1 + r0:1 + r0 + RG],
                                     w4v[:, :, r0:r0 + RG])
                # final: f32 = tree * 2^it + c32 (in place, padded interior)
                nc.vector.scalar_tensor_tensor(
                    fs3[:, :, 1 + r0:1 + r0 + RG],
                    a[:, :].rearrange("p (c r) -> p c r", r=RG),
                    float(2.0 ** it), c32v, OP.mult, OP.add)
                if it != times - 1:
                    s = float(2.0 ** -(it + 1))
                    fb_copies(nfb, s, g)
                    fb_halo_dmas(nfb, g)
                    fbs_copy(nfbs, s, g)
                    fbs_halo_dmas(nfbs, g)
                else:
                    # overlap the store with the other group's compute
                    nc.sync.dma_start(out_v[:, :, r0:r0 + RG],
                                      fs3[:, :, 1 + r0:1 + r0 + RG])

        if times == 0:
            nc.sync.dma_start(out_v, fs3[:, :, 1:1 + H])

    nc.compile()
    nc.m = get_hw_module(nc.m)
    return nc


_CACHE = {}


def _get(times: int):
    if times not in _CACHE:
        _CACHE[times] = _build(times)
    return _CACHE[times]


def kernel(affinity, feature, times, _trace=False, _trace_kwargs=None):
    t = int(times)
    nc = _get(t)
    # transpose to column-major [W, H] on host
    aff = np.ascontiguousarray(
        np.asarray(affinity, dtype=np.float32).transpose(0, 1, 3, 2))
    fea = np.ascontiguousarray(
        np.asarray(feature, dtype=np.float32).transpose(0, 1, 3, 2))
    in_maps = [
        {"affinity": aff[b].reshape(CH, W * H), "feature": fea[b, 0].ravel()}
        for b in range(B)
    ]
    res = bass_utils.run_bass_kernel_spmd(
        nc, in_maps, core_ids=list(range(B)),
        trace=_trace, **(_trace_kwargs or {}),
    )
    out = np.stack([res.results[b]["out"].reshape(W, H).T for b in range(B)])
    out = np.ascontiguousarray(out)[:, None].astype(np.float32)
    if _trace:
        return out, res
    return out


# revision 27
# speedup vs baseline: 1.1807x; 1.1807x over previous
"""AffinityPropagate prediction kernel for Trainium2 (8 NeuronCores).

Data-parallel over batch B=8: each core owns one image [480, 640].

Column-major layout per core: host transposes images to [W=640, H=480];
128 partitions x 5 columns each (exact fit). Each column slot is padded
to 482 (zero row at both ends). State kept in fp32 (single in-place
buffer, own columns only). Each iteration builds scaled fp16 copies fb
(and fbs = fb shifted one element) -- double-buffered -- with left/right
halo column slots refreshed by SBUF->SBUF DMA.

Per iteration (matches reference math):
  f_new = w_center*f + sum_{8 taps} w_t * shift_t(f)
Center product + final add run in fp32 (dominates accuracy); the 8
other products and their pairwise-tree sum run in fp16 at DVE 2x mode,
on values scaled by 2^-iter to stay inside fp16 range (|w_t| <= 1
guarantees no overflow; the unscale folds into the final fused op).
Ops are emitted in 2 row-groups so ACT copies + halo DMAs overlap DVE.
"""

import numpy as np
from contextlib import ExitStack

import concourse.bacc as bacc
import concourse.mybir as mybir
import concourse.tile as tile
from concourse import bass_utils
from concourse.bass_interp import get_hw_module

B, CH, H, W = 8, 8, 480, 640
P = 128            # partitions (each holds COLS_P image columns)
COLS_P = W // P    # 5
CSLOT = H + 2      # col slot: [0, rows at 1..480, 0]
NSLOT = COLS_P + 2  # fb/fbs: + left/right halo column slots
PX = COLS_P * H    # 2400 compact px per partition
RG = H // 2        # row-group size (240)

F32 = mybir.dt.float32
FP16 = mybir.dt.float16
AF = mybir.ActivationFunctionType
OP = mybir.AluOpType

# 8 non-center taps in reference slab order: (dr, dc); w8b slab i = TAPS8[i]
TAPS8 = [(t // 3 - 1, t % 3 - 1) for t in range(9) if t != 4]


def _build(times: int):
    nc = bacc.Bacc("TRN2", debug=False, dynamic_dma_scratch_size=2048)
    # host passes column-major (transposed) data
    aff_d = nc.dram_tensor("affinity", [CH, W * H], F32, kind="ExternalInput")
    feat_d = nc.dram_tensor("feature", [W * H], F32, kind="ExternalInput")
    out_d = nc.dram_tensor("out", [W * H], F32, kind="ExternalOutput")

    with tile.TileContext(nc) as tc, ExitStack() as ctx:
        pool = ctx.enter_context(tc.tile_pool(name="main", bufs=1))

        w8b = pool.tile([P, 8, PX], FP16)          # fp16 tap weights
        w4f = pool.tile([P, PX], F32)              # fp32 center weight
        f32s = pool.tile([P, COLS_P * CSLOT], F32)  # fp32 state (in-place)
        fbp = [pool.tile([P, NSLOT * CSLOT], FP16, name=f"fb{i}")
               for i in range(2)]
        # fp16 scratch for products/tree, per row-group
        pr = [[pool.tile([P, COLS_P * RG], FP16, name=f"pr{g}_{i}")
               for i in range(4)] for g in range(2)]
        sums = pool.tile([P, PX], F32)
        suma = pool.tile([P, PX], F32)
        rec = pool.tile([P, PX], F32)
        rec16 = pool.tile([P, PX], FP16)
        # scr shares slots with the per-iteration c32 ring
        scr = pool.tile([P, PX], F32, tag="c32", bufs=2)

        def f3(t, w):
            return t[:, :].rearrange("p (s x) -> p s x", x=w)

        fs3 = f3(f32s, CSLOT)                      # [P, 5, 482]
        fbv = [f3(t, CSLOT) for t in fbp]          # [P, 7, 482]

        # ---- zero-init padded buffers ----
        nc.vector.memset(f32s[:, :], 0.0)
        for t in fbp:
            nc.vector.memset(t[:, :], 0.0)

        # ---- load feature (transposed [W,H] -> [P,5,480]) ----
        feat_v = feat_d[:].rearrange("(p c r) -> p c r", c=COLS_P, r=H)
        nc.sync.dma_start(fs3[:, :, 1:1 + H], feat_v)

        # ---- load affinity (channel at a time) + weight prep ----
        aff_v = aff_d[:, :].rearrange("c (p x) -> c p x", x=PX)
        for c in range(CH):
            st = pool.tile([P, PX], F32, name=f"stage{c}", tag="stg", bufs=2)
            nc.sync.dma_start(st[:, :], aff_v[c])
            # fp16 copy of channel into weight slab (normalized later)
            nc.scalar.activation(w8b[:, c, :], st[:, :], AF.Copy)
            if c == 0:
                nc.scalar.activation(sums[:, :], st[:, :], AF.Abs)
                nc.vector.tensor_copy(suma[:, :], st[:, :])
            else:
                absdst = rec if c % 2 else w4f
                nc.scalar.activation(absdst[:, :], st[:, :], AF.Abs)
                nc.vector.tensor_add(sums[:, :], sums[:, :], absdst[:, :])
                nc.vector.tensor_add(suma[:, :], suma[:, :], st[:, :])
        nc.vector.reciprocal_approx_accurate(rec[:, :], sums[:, :], scr[:, :])
        # fp16 copy of rec so slab normalization runs in DVE 2x mode
        nc.scalar.activation(rec16[:, :], rec[:, :], AF.Copy)
        for i in range(8):
            nc.vector.tensor_mul(w8b[:, i, :], w8b[:, i, :], rec16[:, :])
        # center = 1 - suma * rec  (fp32)
        nc.vector.scalar_tensor_tensor(w4f[:, :], suma[:, :], -1.0, rec[:, :],
                                       OP.mult, OP.mult)
        nc.vector.tensor_scalar_add(w4f[:, :], w4f[:, :], 1.0)

        def fb_copies(dfb, scale, g):
            # fp16 scaled copy of f32 state rows of group g into dfb
            r0 = g * RG
            nc.scalar.activation(dfb[:, 1:1 + COLS_P, 1 + r0:1 + r0 + RG],
                                 fs3[:, :, 1 + r0:1 + r0 + RG], AF.Copy,
                                 scale=scale)

        def fb_halo_dmas(dfb, g):
            # two HWDGE rings (sync + scalar) to avoid head-of-line blocking
            r0 = g * RG
            sl = slice(1 + r0, 1 + r0 + RG)
            nc.sync.dma_start(dfb[1:P, 0, sl], dfb[0:P - 1, COLS_P, sl])
            nc.scalar.dma_start(dfb[0:P - 1, NSLOT - 1, sl], dfb[1:P, 1, sl])

        for g in (0, 1):
            fb_copies(fbv[0], 1.0, g)
            fb_halo_dmas(fbv[0], g)

        wv = w8b[:, :, :].rearrange("p s (c r) -> p s c r", r=H)
        w4v = w4f[:, :].rearrange("p (c r) -> p c r", r=H)
        out_v = out_d[:].rearrange("(p c r) -> p c r", c=COLS_P, r=H)

        # ---- iterations ----
        for it in range(times):
            cfb = fbv[it % 2]
            nfb = fbv[(it + 1) % 2]
            for g in (0, 1):
                r0 = g * RG
                a, b, c_, d = pr[g]

                def mul8(dst, k):
                    # dr==0 taps are 2B-misaligned -> DVE falls back to 1x;
                    # cheaper than maintaining a shifted copy + its DMAs
                    dr, dc = TAPS8[k]
                    wvg = wv[:, k, :, r0:r0 + RG]
                    src = cfb[:, 1 + dc:1 + dc + COLS_P,
                              1 + r0 + dr:1 + r0 + dr + RG]
                    nc.vector.tensor_mul(
                        dst[:, :].rearrange("p (c r) -> p c r", r=RG),
                        src, wvg)

                def add2(dst, x, y):
                    nc.vector.tensor_add(dst[:, :], x[:, :], y[:, :])

                # halo-free taps (dc==0: k=1,6) first so halo DMAs can
                # complete under them; 1x taps (k=3,4) last
                mul8(a, 1); mul8(b, 6); add2(a, a, b)
                mul8(b, 0); mul8(c_, 2); add2(b, b, c_)
                add2(a, a, b)
                mul8(b, 5); mul8(c_, 7); add2(b, b, c_)
                mul8(c_, 3); mul8(d, 4); add2(c_, c_, d)
                add2(b, b, c_)
                add2(a, a, b)
                # center product fp32: c32 = w4f * f32
                c32 = pool.tile([P, COLS_P * RG], F32, name=f"c32_{it}_{g}",
                                tag="c32", bufs=2)
                c32v = c32[:, :].rearrange("p (c r) -> p c r", r=RG)
                nc.vector.tensor_mul(c32v, fs3[:, :, 1 + r0:1 + r0 + RG],
                                     w4v[:, :, r0:r0 + RG])
                # final: f32 = tree * 2^it + c32 (in place, padded interior)
                nc.vector.scalar_tensor_tensor(
                    fs3[:, :, 1 + r0:1 + r0 + RG],
                    a[:, :].rearrange("p (c r) -> p c r", r=RG),
                    float(2.0 ** it), c32v, OP.mult, OP.add)
                if it != times - 1:
                    s = float(2.0 ** -(it + 1))
                    fb_copies(nfb, s, g)
                    fb_halo_dmas(nfb, g)
                else:
                    # overlap the store with the other group's compute
                    nc.sync.dma_start(out_v[:, :, r0:r0 + RG],
                                      fs3[:, :, 1 + r0:1 + r0 + RG])

        if times == 0:
            nc.sync.dma_start(out_v, fs3[:, :, 1:1 + H])

    nc.compile()
    nc.m = get_hw_module(nc.m)
    return nc


_CACHE = {}


def _get(times: int):
    if times not in _CACHE:
        _CACHE[times] = _build(times)
    return _CACHE[times]


def kernel(affinity, feature, times, _trace=False, _trace_kwargs=None):
    t = int(times)
    nc = _get(t)
    # transpose to column-major [W, H] on host
    aff = np.ascontiguousarray(
        np.asarray(affinity, dtype=np.float32).transpose(0, 1, 3, 2))
    fea = np.ascontiguousarray(
        np.asarray(feature, dtype=np.float32).transpose(0, 1, 3, 2))
    in_maps = [
        {"affinity": aff[b].reshape(CH, W * H), "feature": fea[b, 0].ravel()}
        for b in range(B)
    ]
    res = bass_utils.run_bass_kernel_spmd(
        nc, in_maps, core_ids=list(range(B)),
        trace=_trace, **(_trace_kwargs or {}),
    )
    out = np.stack([res.results[b]["out"].reshape(W, H).T for b in range(B)])
    out = np.ascontiguousarray(out)[:, None].astype(np.float32)
    if _trace:
        return out, res
    return out
